# revision 9
# baseline (speedup 1.0000x reference)
"""GATv2 encoder (2-layer, PyG semantics) on 8 TRN2 cores — v3 dst-major.

Layout: nodes partitioned by dst core; per core, destination nodes are
assigned SBUF lanes (partition = dst lane) in TWO orderings — canonical
(sorted by lo-section in-degree) and hi (sorted by hi-section in-degree) —
so each gather section packs tightly (pad ~10% vs ~70% for a single
ordering). Per chunk of 128 dst lanes, incoming-edge source rows are
dma_gathered from the AllGathered |a|-folded source table; the target-side
transform adds via a lane-broadcast; per-head sign blocks of the lrelu'd
sum reduce directly to logits (attention sign folded as a host-side column
permutation: logit = sum_pos lrelu - sum_neg lrelu); exp weights are
written pair-duplicated so the weighted-feature multiply stays on the DVE
2x path; numerator+denominator come from one strided d-reduction. The
hi-ordering partial sums realign to canonical lanes with one dma_gather.

Layer 1's source table/target transforms depend only on inputs, so the
host stages them directly (no AG, no table phase); layer 2 computes its
tables on device from h and AllGathers the source table.
"""
import numpy as np
import ml_dtypes

try:
    import concourse  # noqa: F401
except ImportError:  # pragma: no cover
    import sys
    sys.path.insert(0, "/opt/trn_rl_repo")

from concourse import bass, bacc, mybir, tile
from concourse import bass_utils

F32 = mybir.dt.float32
BF16 = mybir.dt.bfloat16
I16 = mybir.dt.int16
NPBF = ml_dtypes.bfloat16

N_NODES = 50000
N_CORES = 8
FEAT = 128
HEADS1 = 4
NEG = -75.0        # mask logit offset: exp(-75) ~ 2.6e-33


class Cfg:
    def __init__(self):
        self.N = N_NODES
        self.NC = N_CORES
        self.NPC = self.N // self.NC
        self.P = 128
        self.CH = (self.NPC + 127) // 128          # 49
        self.SLOTS = self.CH * 128                 # 6272
        self.TOT = self.SLOTS * self.NC            # 50176
        self.HALF = 5 * self.SLOTS                 # 31360 < 32768
        self.F = FEAT
        self.H1 = HEADS1
        self.queues = 4
        self.maxt = 8          # tiles per dma_gather
        self.wp_bufs = 3
        self.sp_bufs = 4
        self.pp_bufs = 2
        self.RG = 7            # realign chunk group size


# ---------------------------------------------------------------- host prep

def prep_weights3(att, Wl, bl, Wr, br, bias, prev_perm=None):
    """|a|-fold + per-head [pos|neg] column permutation.

    logit = sum_c sign_c * lrelu(|a_c| u_c); with columns permuted so each
    head is [pos block | neg block], logit = sum_pos lrelu - sum_neg lrelu.
    inva carries 2/|a| (pair-duplicated denominators sum to 2*sum w).
    """
    H, C = att.shape
    a = att.reshape(-1).astype(np.float64)
    absa = np.maximum(np.abs(a), 1e-12)
    perm = []
    cpos = []
    for h in range(H):
        cols = np.arange(h * C, (h + 1) * C)
        pos = cols[a[cols] >= 0]
        neg = cols[a[cols] < 0]
        perm.extend(pos.tolist() + neg.tolist())
        cpos.append(len(pos))
    perm = np.array(perm, dtype=np.int64)
    Wl2 = (Wl.astype(np.float64) * absa[None, :])[:, perm]
    Wr2 = (Wr.astype(np.float64) * absa[None, :])[:, perm]
    bl2 = (bl.astype(np.float64) * absa)[perm]
    br2 = (br.astype(np.float64) * absa)[perm]
    if prev_perm is not None:
        Wl2 = Wl2[prev_perm, :]
        Wr2 = Wr2[prev_perm, :]
    return dict(
        Wl=Wl2.astype(np.float32), bl=bl2.astype(np.float32),
        Wr=Wr2.astype(np.float32), br=br2.astype(np.float32),
        inva=(1.0 / absa[perm]).astype(np.float32),
        bias=bias.astype(np.float32)[perm],
        perm=perm, cpos=cpos, H=H, C=C,
    )


def _wrap16(ids):
    a = np.asarray(ids, dtype=np.int16).reshape(-1, 16).T
    return np.tile(a, (8, 1))


def _lane_table(lane, val, nlanes, width):
    """mat[lane, i] = i-th val of that lane (order of appearance)."""
    order = np.argsort(lane, kind="stable")
    ls = lane[order]
    vs = val[order]
    cnt = np.bincount(ls, minlength=nlanes)
    starts = np.concatenate([[0], np.cumsum(cnt)[:-1]])
    within = np.arange(len(ls)) - starts[ls]
    mat = np.zeros((nlanes, width), dtype=np.int64)
    mat[ls, within] = vs
    return mat, cnt


def prep_graph3(edge_index, cfg):
    N, NPC, NC, P, CH, SLOTS, HALF = (cfg.N, cfg.NPC, cfg.NC, cfg.P, cfg.CH,
                                      cfg.SLOTS, cfg.HALF)
    src = np.concatenate([np.asarray(edge_index[0], np.int64),
                          np.arange(N, dtype=np.int64)])
    dst = np.concatenate([np.asarray(edge_index[1], np.int64),
                          np.arange(N, dtype=np.int64)])
    lo_edge = (src // NPC) < 5

    cores = []
    newid = np.full(N, -1, dtype=np.int64)
    for c in range(NC):
        m = (dst >= c * NPC) & (dst < (c + 1) * NPC)
        s_c = src[m]
        d_c = dst[m] - c * NPC
        lo_c = lo_edge[m]
        dlo = np.bincount(d_c[lo_c], minlength=NPC)
        dhi = np.bincount(d_c[~lo_c], minlength=NPC)
        ord_lo = np.argsort(-dlo, kind="stable")
        ord_hi = np.argsort(-dhi, kind="stable")
        slot_lo = np.empty(NPC, np.int64)
        slot_lo[ord_lo] = np.arange(NPC)
        slot_hi = np.empty(NPC, np.int64)
        slot_hi[ord_hi] = np.arange(NPC)
        newid[c * NPC:(c + 1) * NPC] = c * SLOTS + slot_lo
        cores.append(dict(s=s_c, d=d_c, lo=lo_c, dlo=dlo, dhi=dhi,
                          ord_lo=ord_lo, ord_hi=ord_hi,
                          slot_lo=slot_lo, slot_hi=slot_hi))

    # global per-chunk widths: degrees in canonical/hi order are sorted
    # descending, so a chunk's max lane degree is its first lane's degree
    D_LO = np.zeros(CH, np.int64)
    D_HI = np.zeros(CH, np.int64)
    for c in range(NC):
        dlo_s = np.concatenate([cores[c]["dlo"][cores[c]["ord_lo"]],
                                np.zeros(SLOTS - NPC, np.int64)])
        dhi_s = np.concatenate([cores[c]["dhi"][cores[c]["ord_hi"]],
                                np.zeros(SLOTS - NPC, np.int64)])
        D_LO = np.maximum(D_LO, dlo_s.reshape(CH, P).max(1))
        D_HI = np.maximum(D_HI, dhi_s.reshape(CH, P).max(1))
    D_LO = np.maximum(D_LO, 1)
    D_HI = np.maximum(D_HI, 1)

    out = []
    for c in range(NC):
        cc = cores[c]
        sid = newid[cc["s"]]
        res = {}
        for sec, selm, slots_of, Ds, base in (
                ("lo", cc["lo"], cc["slot_lo"], D_LO, 0),
                ("hi", ~cc["lo"], cc["slot_hi"], D_HI, HALF)):
            lane = slots_of[cc["d"][selm]]
            vals = sid[selm] - base
            W = int(Ds.max())
            mat, cnt = _lane_table(lane, vals, SLOTS, W)
            cnt_mat = cnt.reshape(CH, P)
            idx_parts, mask_parts = [], []
            for g in range(CH):
                Dg = int(Ds[g])
                sub = mat[g * P:(g + 1) * P, :Dg]      # [128, Dg]
                idx_parts.append(_wrap16(sub.T.reshape(-1)))
                msk = np.where(np.arange(Dg)[None, :] < cnt_mat[g][:, None],
                               0.0, NEG)
                mask_parts.append(msk.astype(np.float32))
            res[f"idx_{sec}"] = np.concatenate(idx_parts, axis=1)
            res[f"mask_{sec}"] = np.concatenate(mask_parts, axis=1)
        # realign: canonical slot s -> hi slot of its node (0 if empty)
        ral = np.zeros(SLOTS, np.int64)
        ral[:NPC] = cc["slot_hi"][cc["ord_lo"]]
        res["ral_idx"] = _wrap16(ral)
        # vr-hi: hi slot q -> canonical slot of its node (0 if empty)
        vrh = np.zeros(SLOTS, np.int64)
        vrh[:NPC] = cc["slot_lo"][cc["ord_hi"]]
        res["vrh_idx"] = _wrap16(vrh)
        res["ord_lo"] = cc["ord_lo"]
        res["ord_hi"] = cc["ord_hi"]
        res["slot_lo"] = cc["slot_lo"]
        out.append(res)
    return out, D_LO, D_HI


# ---------------------------------------------------------------- device

def declare_io(nc, cfg, SDLO, SDHI):
    P, F, CH, SLOTS, TOT = cfg.P, cfg.F, cfg.CH, cfg.SLOTS, cfg.TOT
    d = {}
    def inp(name, shape, dt):
        d[name] = nc.dram_tensor(name, list(shape), dt,
                                 kind="ExternalInput").ap()
    inp("tab1", (TOT, F), BF16)
    inp("vr1", (P, CH * F), BF16)
    inp("vr1hi", (P, CH * F), BF16)
    inp("W2lr", (P, 2 * F), BF16)
    inp("bb2lr", (P, 2 * F), F32)
    for n in ("inva1", "gbias1", "inva2", "gbias2"):
        inp(n, (P, F), F32)
    inp("identb", (P, P), BF16)
    inp("idx_lo", (P, 8 * SDLO), I16)
    inp("idx_hi", (P, 8 * SDHI), I16)
    inp("mask_lo", (P, SDLO), F32)
    inp("mask_hi", (P, SDHI), F32)
    inp("ral_idx", (P, CH * 8), I16)
    inp("vrh_idx", (P, CH * 8), I16)
    d["out"] = nc.dram_tensor("out", [SLOTS, F], F32,
                              kind="ExternalOutput").ap()
    d["hdbg"] = nc.dram_tensor("hdbg", [SLOTS, F], BF16,
                               kind="ExternalOutput").ap()
    d["pldbg"] = nc.dram_tensor("pldbg", [P, CH * 136], F32,
                                kind="ExternalOutput").ap()
    d["phdbg"] = nc.dram_tensor("phdbg", [SLOTS, 192], F32,
                                kind="ExternalOutput").ap()
    return d


def build_program(tc, io, cfg, D_LO, D_HI, cpos1, cpos2):
    nc = tc.nc
    P, F, CH, SLOTS, TOT, HALF = (cfg.P, cfg.F, cfg.CH, cfg.SLOTS, cfg.TOT,
                                  cfg.HALF)
    H1, H2 = cfg.H1, 1
    MAXT, RG = cfg.maxt, cfg.RG
    MAXW = F + 2 * H1                      # widest aug row (layer 1)
    DLM = int(D_LO.max())
    DHM = int(D_HI.max())
    offs_lo = np.concatenate([[0], np.cumsum(D_LO)]).astype(int)
    offs_hi = np.concatenate([[0], np.cumsum(D_HI)]).astype(int)
    qctr = [0]

    with (
        tc.tile_pool(name="consts", bufs=1) as cpool,
        tc.tile_pool(name="work", bufs=cfg.wp_bufs) as wp,
        tc.tile_pool(name="small", bufs=cfg.sp_bufs) as sp,
        tc.tile_pool(name="psum", bufs=cfg.pp_bufs, space="PSUM") as pp,
        tc.tile_pool(name="dram", bufs=1, space="DRAM") as dp,
    ):
        C = {}
        def load_const(name, shape, dt):
            t = cpool.tile(list(shape), dt, tag=name)
            nc.sync.dma_start(t[:], io[name])
            C[name] = t
            return t
        load_const("W2lr", (P, 2 * F), BF16)
        load_const("bb2lr", (P, 2 * F), F32)
        for n in ("inva1", "gbias1", "inva2", "gbias2"):
            load_const(n, (P, F), F32)
        load_const("identb", (P, P), BF16)
        load_const("idx_lo", (P, 8 * int(offs_lo[-1])), I16)
        load_const("idx_hi", (P, 8 * int(offs_hi[-1])), I16)
        load_const("mask_lo", (P, int(offs_lo[-1])), F32)
        load_const("mask_hi", (P, int(offs_hi[-1])), F32)
        load_const("ral_idx", (P, CH * 8), I16)
        load_const("vrh_idx", (P, CH * 8), I16)
        load_const("vr1", (P, CH * F), BF16)
        load_const("vr1hi", (P, CH * F), BF16)

        h_sb = cpool.tile([P, CH * F], BF16, tag="h_sb")
        vl2_sb = cpool.tile([P, CH * F], BF16, tag="vl2_sb")
        vr2_sb = C["vr1"]          # reuse: vr1 dead after layer 1
        vr2hi_sb = C["vr1hi"]
        PL_sb = cpool.tile([P, CH * MAXW], F32, tag="PL")

        vl2_dram = dp.tile([SLOTS, F], BF16)
        vr2_dram = dp.tile([SLOTS, F], BF16)
        ag_space = ("Shared" if cfg.NC > 1
                    and not getattr(cfg, "sim_fake_ag", False) else "Local")
        tab2 = dp.tile([TOT, F], BF16, addr_space=ag_space)
        PH_dram1 = dp.tile([SLOTS, 192], F32)
        PH_dram2 = dp.tile([SLOTS, 192], F32)

        def gathers(dst3, tab_ap, idx_sb, off8, D):
            for a in range(0, D, MAXT):
                b = min(a + MAXT, D)
                nc.gpsimd.dma_gather(
                    out_ap=dst3[:, a:b, :], in_ap=tab_ap,
                    idxs_ap=idx_sb[:, off8 + a * 8: off8 + b * 8],
                    num_idxs=(b - a) * P, num_idxs_reg=(b - a) * P,
                    elem_size=F, queue_num=qctr[0] % cfg.queues,
                    single_packet=True)
                qctr[0] += 1

        def edge_half(g, sec, tab, vr_sb, H, Cp, dest, dest_col):
            """One chunk of one section -> partial sums [P, F+2H] f32."""
            Cc = F // H
            D = int((D_LO if sec == "lo" else D_HI)[g])
            off = int((offs_lo if sec == "lo" else offs_hi)[g])
            DM = DLM if sec == "lo" else DHM
            idx_sb = C["idx_lo" if sec == "lo" else "idx_hi"]
            mask_sb = C["mask_lo" if sec == "lo" else "mask_hi"]
            W2 = F + 2 * H

            ul = wp.tile([P, DM * F], BF16, tag=f"ul{sec}")
            ul3 = ul[:, 0:D * F].rearrange("p (d f) -> p d f", f=F)
            gathers(ul3, tab, idx_sb, off * 8, D)

            # aug doubles as lrelu scratch: cols 0:F hold lrelu(ul+vr) until
            # the weighted multiply overwrites them (frees a work tile so the
            # pool can triple-buffer chunks)
            aug = wp.tile([P, DM * MAXW], BF16, tag=f"aug{sec}")
            aug3 = aug[:, 0:D * W2].rearrange("p (d c) -> p d c", c=W2)
            av = aug3[:, :, 0:F]
            nc.vector.tensor_tensor(
                out=av, in0=ul3,
                in1=vr_sb[:, g * F:(g + 1) * F]
                    .rearrange("p (o f) -> p o f", o=1)
                    .to_broadcast([P, D, F]),
                op=mybir.AluOpType.add)
            if getattr(cfg, "sim_safe", False):
                # CoreSim has no Prelu; DVE equivalent for exec-sim checks
                t02 = wp.tile([P, DM * F], BF16, tag=f"t02{sec}", bufs=1)
                t023 = t02[:, 0:D * F].rearrange("p (d f) -> p d f", f=F)
                nc.vector.tensor_scalar(out=t023, in0=av, scalar1=0.2,
                                        scalar2=None,
                                        op0=mybir.AluOpType.mult)
                nc.vector.tensor_tensor(out=av, in0=av, in1=t023,
                                        op=mybir.AluOpType.max)
            else:
                nc.scalar.activation(out=av, in_=av,
                                     func=mybir.ActivationFunctionType.Prelu,
                                     alpha=0.2)
            lr4 = aug3[:, :, 0:F].rearrange("p d (h c) -> p d h c", h=H)

            # logits: per-head [pos|neg] block reduces
            rp = sp.tile([P, DLM * H1], F32, tag=f"rp{sec}")
            rn = sp.tile([P, DLM * H1], F32, tag=f"rn{sec}")
            rp4 = rp[:, 0:D * H].rearrange("p (d h o) -> p d h o", h=H, o=1)
            rn4 = rn[:, 0:D * H].rearrange("p (d h o) -> p d h o", h=H, o=1)
            if any(cp == 0 for cp in Cp):
                nc.vector.memset(rp[:, 0:D * H], 0.0)
            if any(cp == Cc for cp in Cp):
                nc.vector.memset(rn[:, 0:D * H], 0.0)
            for h in range(H):
                cp = Cp[h]
                if cp > 0:
                    nc.vector.tensor_reduce(
                        out=rp4[:, :, h:h + 1, :],
                        in_=lr4[:, :, h:h + 1, 0:cp],
                        axis=mybir.AxisListType.X, op=mybir.AluOpType.add)
                if cp < Cc:
                    nc.vector.tensor_reduce(
                        out=rn4[:, :, h:h + 1, :],
                        in_=lr4[:, :, h:h + 1, cp:Cc],
                        axis=mybir.AxisListType.X, op=mybir.AluOpType.add)
            logit = sp.tile([P, DLM * H1], F32, tag=f"lg{sec}")
            nc.vector.tensor_tensor(out=logit[:, 0:D * H], in0=rp[:, 0:D * H],
                                    in1=rn[:, 0:D * H],
                                    op=mybir.AluOpType.subtract)
            lg3 = logit[:, 0:D * H].rearrange("p (d h) -> p d h", h=H)
            nc.vector.tensor_tensor(
                out=lg3, in0=lg3,
                in1=mask_sb[:, off:off + D]
                    .rearrange("p (d o) -> p d o", o=1)
                    .to_broadcast([P, D, H]),
                op=mybir.AluOpType.add)

            # w = exp(logit), pair-duplicated into aug cols F:F+2H
            wpr = aug3[:, :, F:F + 2 * H].rearrange(
                "p d (h two) -> p d h two", two=2)
            lg4 = logit[:, 0:D * H].rearrange("p (d h o) -> p d h o",
                                              h=H, o=1)
            nc.scalar.activation(out=wpr[:, :, :, 0:1], in_=lg4,
                                 func=mybir.ActivationFunctionType.Exp)
            nc.scalar.activation(out=wpr[:, :, :, 1:2], in_=lg4,
                                 func=mybir.ActivationFunctionType.Exp)
            # ISA allows <=3 free AP dims: one pair-trick multiply per head
            for h in range(H):
                nc.vector.tensor_tensor(
                    out=aug3[:, :, h * Cc:(h + 1) * Cc].rearrange(
                        "p d (c2 two) -> p d c2 two", two=2),
                    in0=ul3[:, :, h * Cc:(h + 1) * Cc].rearrange(
                        "p d (c2 two) -> p d c2 two", two=2),
                    in1=aug3[:, :, F + 2 * h:F + 2 * h + 2].rearrange(
                        "p d (o two) -> p d o two", o=1, two=2)
                        .to_broadcast([P, D, Cc // 2, 2]),
                    op=mybir.AluOpType.mult)

            # partial sums over d
            if getattr(cfg, "tree_final", False):
                # in-place contiguous bf16 fold tree (HW dislikes the
                # fully-strided reduce), then one f32 convert into dest
                cur = D
                while cur > 1:
                    half = cur // 2
                    rest = cur - half
                    nc.vector.tensor_tensor(
                        out=aug[:, 0:half * W2], in0=aug[:, 0:half * W2],
                        in1=aug[:, rest * W2:cur * W2],
                        op=mybir.AluOpType.add)
                    cur = rest
                nc.vector.tensor_copy(
                    out=dest[:, dest_col:dest_col + W2], in_=aug[:, 0:W2])
            else:
                nc.vector.tensor_reduce(
                    out=dest[:, dest_col:dest_col + W2]
                        .rearrange("p (c o) -> p c o", o=1),
                    in_=aug[:, 0:D * W2].rearrange("p (d c) -> p c d", c=W2),
                    axis=mybir.AxisListType.X, op=mybir.AluOpType.add)

        def combine_norm(g, phc, pcol, H, inva, gbias, elu, out_rows):
            W2 = F + 2 * H
            o = sp.tile([P, MAXW], F32, tag="o")
            nc.vector.tensor_tensor(out=o[:, 0:W2],
                                    in0=PL_sb[:, g * W2:(g + 1) * W2],
                                    in1=phc[:, pcol:pcol + W2],
                                    op=mybir.AluOpType.add)
            rec = sp.tile([P, H1], F32, tag="rec")
            nc.vector.reciprocal(
                rec[:, 0:H].rearrange("p (h o) -> p h o", o=1),
                o[:, F:F + 2 * H].rearrange("p (h two) -> p h two",
                                            two=2)[:, :, 0:1])
            t1 = sp.tile([P, F], F32, tag="t1")
            nc.vector.tensor_tensor(
                out=t1[:].rearrange("p (h c) -> p h c", h=H),
                in0=inva[:].rearrange("p (h c) -> p h c", h=H),
                in1=rec[:, 0:H].rearrange("p (h o) -> p h o", o=1)
                    .to_broadcast([P, H, F // H]),
                op=mybir.AluOpType.mult)
            o1 = sp.tile([P, F], F32, tag="o1")
            nc.vector.tensor_tensor(out=o1[:], in0=o[:, 0:F], in1=t1[:],
                                    op=mybir.AluOpType.mult)
            nc.vector.tensor_tensor(out=o1[:], in0=o1[:], in1=gbias[:],
                                    op=mybir.AluOpType.add)
            if elu:
                m0 = sp.tile([P, F], F32, tag="m0")
                nc.vector.tensor_scalar(out=m0[:], in0=o1[:], scalar1=0.0,
                                        scalar2=None,
                                        op0=mybir.AluOpType.min)
                e0 = sp.tile([P, F], F32, tag="e0")
                nc.scalar.activation(out=e0[:], in_=m0[:],
                                     func=mybir.ActivationFunctionType.Exp)
                nc.vector.scalar_tensor_tensor(
                    out=o1[:], in0=o1[:], scalar=0.0, in1=e0[:],
                    op0=mybir.AluOpType.max, op1=mybir.AluOpType.add)
                nc.scalar.activation(out=h_sb[:, g * F:(g + 1) * F],
                                     in_=o1[:],
                                     func=mybir.ActivationFunctionType.Copy,
                                     bias=-1.0)
            else:
                nc.sync.dma_start(out_rows[g * P:(g + 1) * P, :], o1[:])

        def edge_layer(tab, vr_can, vr_hi, H, Cp, inva, gbias, PH_dram,
                       elu, out_rows):
            W2 = F + 2 * H
            # hi section first: its realign round trip (a whole-layer
            # barrier) then overlaps the lo-section compute
            for g in range(CH):
                ph = sp.tile([P, 192], F32, tag="ph", bufs=3)
                nc.vector.memset(ph[:, W2:192], 0.0)
                edge_half(g, "hi", tab[HALF:TOT, :], vr_hi, H, Cp, ph, 0)
                nc.sync.dma_start(
                    PH_dram[:].rearrange("(g p) c -> p g c", p=P)
                    [:, g:g + 1, :],
                    ph[:].rearrange("p (o c) -> p o c", o=1))
            phcs = []
            for j in range((CH + RG - 1) // RG):
                g0 = j * RG
                g1 = min(g0 + RG, CH)
                ng = g1 - g0
                phc = sp.tile([P, RG * 192], F32, tag=f"phc{j % 2}", bufs=1)
                phc3 = phc[:, 0:ng * 192].rearrange("p (g c) -> p g c", c=192)
                nc.gpsimd.dma_gather(
                    out_ap=phc3, in_ap=PH_dram[:, :],
                    idxs_ap=C["ral_idx"][:, g0 * 8:g1 * 8],
                    num_idxs=ng * P, num_idxs_reg=ng * P,
                    elem_size=192, queue_num=qctr[0] % cfg.queues,
                    single_packet=True)
                qctr[0] += 1
                phcs.append(phc)
            for g in range(CH):
                edge_half(g, "lo", tab[0:HALF, :], vr_can, H, Cp,
                          PL_sb, g * W2)
                combine_norm(g, phcs[g // RG], (g % RG) * 192, H, inva,
                             gbias, elu, out_rows)

        # ---------------- layer 1 (host-staged tables)
        edge_layer(io["tab1"], C["vr1"], C["vr1hi"], H1, cpos1,
                   C["inva1"], C["gbias1"], PH_dram1, elu=True,
                   out_rows=None)

        if getattr(cfg, "debug_h", False):
            nc.sync.dma_start(
                io["hdbg"].rearrange("(g p) f -> p g f", p=P),
                h_sb[:].rearrange("p (g f) -> p g f", f=F))
            nc.sync.dma_start(io["pldbg"], PL_sb[:])
            nc.sync.dma_start(io["phdbg"], PH_dram1[:, :])

        # ---------------- table phase 2 (h -> vl2/vr2)
        for g in range(CH):
            ps_t = pp.tile([P, P], BF16, tag="pst")
            nc.tensor.transpose(out=ps_t[:], in_=h_sb[:, g * F:(g + 1) * F],
                                identity=C["identb"][:])
            hT = sp.tile([P, P], BF16, tag="hT")
            nc.vector.tensor_copy(out=hT[:], in_=ps_t[:])
            ps_lr = pp.tile([P, 2 * F], F32, tag="pslr")
            nc.tensor.matmul(ps_lr[:], lhsT=hT[:], rhs=C["W2lr"][:],
                             start=True, stop=True)
            nc.vector.tensor_tensor(out=vl2_sb[:, g * F:(g + 1) * F],
                                    in0=ps_lr[:, 0:F], in1=C["bb2lr"][:, 0:F],
                                    op=mybir.AluOpType.add)
            nc.vector.tensor_tensor(out=vr2_sb[:, g * F:(g + 1) * F],
                                    in0=ps_lr[:, F:2 * F],
                                    in1=C["bb2lr"][:, F:2 * F],
                                    op=mybir.AluOpType.add)
        nc.sync.dma_start(
            vl2_dram[:].rearrange("(g p) f -> p g f", p=P),
            vl2_sb[:].rearrange("p (g f) -> p g f", f=F))
        nc.sync.dma_start(
            vr2_dram[:].rearrange("(g p) f -> p g f", p=P),
            vr2_sb[:].rearrange("p (g f) -> p g f", f=F))

        # AllGather vl2 -> tab2
        if getattr(cfg, "sim_fake_ag", False):
            for c in range(cfg.NC):
                nc.sync.dma_start(tab2[c * SLOTS:(c + 1) * SLOTS, :],
                                  vl2_dram[0:SLOTS, :])
        elif cfg.NC == 1:
            nc.sync.dma_start(tab2[0:SLOTS, :], vl2_dram[0:SLOTS, :])
        else:
            nc.gpsimd.collective_compute(
                "AllGather", mybir.AluOpType.bypass,
                replica_groups=[list(range(cfg.NC))],
                ins=[vl2_dram[0:SLOTS, :]], outs=[tab2[:, :]])

        # vr2 in hi order via gather
        for j in range((CH + 7) // 8):
            g0, g1 = j * 8, min(j * 8 + 8, CH)
            nc.gpsimd.dma_gather(
                out_ap=vr2hi_sb[:, g0 * F:g1 * F]
                    .rearrange("p (g f) -> p g f", f=F),
                in_ap=vr2_dram[:, :],
                idxs_ap=C["vrh_idx"][:, g0 * 8:g1 * 8],
                num_idxs=(g1 - g0) * P, num_idxs_reg=(g1 - g0) * P,
                elem_size=F, queue_num=qctr[0] % cfg.queues,
                single_packet=True)
            qctr[0] += 1

        # ---------------- layer 2
        edge_layer(tab2[:, :], vr2_sb, vr2hi_sb, H2, cpos2,
                   C["inva2"], C["gbias2"], PH_dram2, elu=False,
                   out_rows=io["out"])


# ---------------------------------------------------------------- runner

_LAST = {}


def _build(inputs, cfg):
    x = np.asarray(inputs["x"], np.float32)
    ei = np.asarray(inputs["edge_index"])
    w1 = prep_weights3(np.asarray(inputs["att1"], np.float32),
                       np.asarray(inputs["W1l"], np.float32),
                       np.asarray(inputs["b1l"], np.float32),
                       np.asarray(inputs["W1r"], np.float32),
                       np.asarray(inputs["b1r"], np.float32),
                       np.asarray(inputs["bias1"], np.float32))
    w2 = prep_weights3(np.asarray(inputs["att2"], np.float32),
                       np.asarray(inputs["W2l"], np.float32),
                       np.asarray(inputs["b2l"], np.float32),
                       np.asarray(inputs["W2r"], np.float32),
                       np.asarray(inputs["b2r"], np.float32),
                       np.asarray(inputs["bias2"], np.float32),
                       prev_perm=w1["perm"])
    grs, D_LO, D_HI = prep_graph3(ei, cfg)
    cfg.D_LO, cfg.D_HI = D_LO, D_HI

    # host-side layer-1 tables (canonical gid order, shared by all cores)
    xl1 = (x @ w1["Wl"] + w1["bl"]).astype(np.float32)
    xr1 = (x @ w1["Wr"] + w1["br"]).astype(np.float32)
    NPC, SLOTS, P, CH, F = cfg.NPC, cfg.SLOTS, cfg.P, cfg.CH, cfg.F
    tab1 = np.zeros((cfg.TOT, F), np.float32)
    for c in range(cfg.NC):
        tab1[c * SLOTS:c * SLOTS + NPC] = xl1[c * NPC + grs[c]["ord_lo"]]
    tab1 = tab1.astype(NPBF)

    rowb = lambda v: np.broadcast_to(v.astype(np.float32), (P, F)).copy()
    # [P, CH*F] lane-major: lane p, chunk g cols -> node at slot g*128+p
    def to_lane(a):
        return np.ascontiguousarray(
            a.reshape(CH, P, F).transpose(1, 0, 2).reshape(P, CH * F))
    in_maps = []
    for c in range(cfg.NC):
        gr = grs[c]
        vr1 = np.zeros((SLOTS, F), np.float32)
        vr1[:NPC] = xr1[c * NPC + gr["ord_lo"]]
        vr1hi = np.zeros((SLOTS, F), np.float32)
        vr1hi[:NPC] = xr1[c * NPC + gr["ord_hi"]]
        in_maps.append({
            "tab1": tab1,
            "vr1": to_lane(vr1).astype(NPBF),
            "vr1hi": to_lane(vr1hi).astype(NPBF),
            "W2lr": np.concatenate([w2["Wl"], w2["Wr"]], axis=1).astype(NPBF),
            "bb2lr": np.concatenate(
                [rowb(w2["bl"]), rowb(w2["br"])], axis=1),
            "inva1": rowb(w1["inva"]), "gbias1": rowb(w1["bias"]),
            "inva2": rowb(w2["inva"]), "gbias2": rowb(w2["bias"]),
            "identb": np.eye(P, dtype=NPBF),
            "idx_lo": gr["idx_lo"], "idx_hi": gr["idx_hi"],
            "mask_lo": gr["mask_lo"].astype(np.float32),
            "mask_hi": gr["mask_hi"].astype(np.float32),
            "ral_idx": gr["ral_idx"], "vrh_idx": gr["vrh_idx"],
        })

    num_dev = 1 if getattr(cfg, "sim_fake_ag", False) else cfg.NC
    nc = bacc.Bacc("TRN2", target_bir_lowering=False, debug=False,
                   num_devices=num_dev, num_swdge_queues=cfg.queues)
    io = declare_io(nc, cfg, int(D_LO.sum()), int(D_HI.sum()))
    with tile.TileContext(nc) as tc:
        build_program(tc, io, cfg, D_LO, D_HI, w1["cpos"], w2["cpos"])
    nc.compile()
    return nc, in_maps, grs, (w1, w2)


def kernel(**inputs) -> np.ndarray:
    cfg = Cfg()
    nc, in_maps, grs, (w1, w2) = _build(inputs, cfg)
    try:
        res = bass_utils.run_bass_kernel_spmd(nc, in_maps,
                                              core_ids=list(range(cfg.NC)))
    except Exception:
        import time
        time.sleep(5)
        res = bass_utils.run_bass_kernel_spmd(nc, in_maps,
                                              core_ids=list(range(cfg.NC)))
    _LAST.update(results=res, nc=nc, in_maps=in_maps, cfg=cfg, grs=grs,
                 w=(w1, w2))

    out = np.zeros((cfg.N, cfg.F), np.float32)
    iperm2 = np.empty(cfg.F, np.int64)
    iperm2[w2["perm"]] = np.arange(cfg.F)
    for c in range(cfg.NC):
        oc = np.asarray(res.results[c]["out"]).reshape(cfg.SLOTS, cfg.F)
        out[c * cfg.NPC:(c + 1) * cfg.NPC] = (
            oc[grs[c]["slot_lo"]][:, iperm2])
    return out
